# revision 26
# baseline (speedup 1.0000x reference)
"""BTSPAttention Trainium2 kernel for 8 NeuronCores (self-contained).

Usage: kernel(**inputs) -> np.ndarray  (full [2,2048,1024] float32 output)

Sharding: 8 cores = 2 batches x 4 head-groups (4 heads each).

v2 dataflow (vs v1 baseline at 546us; this version ~330-390us):
  - bf16 matmul operands everywhere (warm PE streams 2 bf16/cycle).
  - The clipped-Toeplitz time-bias multiply is skipped outside the +-250
    diagonal band: out-of-band [128k, 1024q] tiles fold the constant edge
    bias into the exp activation's bias operand (from a [128,2] SBUF
    table, so the program stays input-independent).
  - bk dropped entirely (contributes only q-constant scores terms, which
    softmax ignores); bq applied via a rank-1 start matmul.
  - Softmax reciprocals: sum rows are DMA-packed [1,1024]->[64,16] so one
    exact DVE reciprocal covers a whole pass at 16 elems/lane, scattered
    back (f32r) for the ones-stationary broadcast matmul.
  - Software pipelining tuned for the PE HAM clock gate (K=8/8 needs
    gap-free streams): a warm-up matmul spin covers the initial x DMA;
    per pass, AV emission lags scores by 4 chunks; each pass's tail AVs,
    normalize chain, and each head's output projection are deferred into
    the next pass's chunk slots (absolute-indexed pend schedule) so no
    engine FIFO ever blocks at a pass boundary.

Per head, per q-half pass (sp, 1024 cols), per k-chunk c (128 wide):
  scoresT[k,q] psum = kT_chunk.T @ qT      (2 N=512 bf16 matmuls)
  P = exp(0.125*scores + bias)             (ACT, [128,1024] per chunk)
  P *= eb[band]                            (DVE, banded chunks only)
  ctx[65, 2si, 512] psum += [V|1].T @ P    (row 64 = softmax sum)
Then: pack/recip/scatter + broadcast matmul + DVE normalize-mul -> ctxn
bf16; out rows are disjoint per head (faithful-torch 5-D transpose):
  y[hl*128 + tc, :] = sum_tf ctxn[:, 16tc+tf].T @ wog[:, tf, :]
Host folds: is_gate dropped (softmax shift-invariance); bv and bo applied
exactly on the host after the gather.
"""

import numpy as np
import ml_dtypes

import sys as _sys
if '/opt/trn_rl_repo' not in _sys.path:
    _sys.path.insert(0, '/opt/trn_rl_repo')


import concourse.bass as bass
import concourse.tile as tile
from concourse import bacc
from concourse import mybir

F32 = mybir.dt.float32
F32R = mybir.dt.float32r
BF16 = mybir.dt.bfloat16
AF = mybir.ActivationFunctionType

T = 2048
D = 1024
HD = 64
TB_LEN = 500
NKC = 16   # k chunks of 128
NDC = 8    # D chunks of 128

# (c, half) pairs whose [128k, 1024q] tile intersects the |k-q|<=250 band.
# half0 (q in [0,1024)): idx = 128c+p-q+250 varies iff c <= 9
# half1 (q in [1024,2048)): varies iff c >= 6
BAND = [(c, h) for c in range(NKC) for h in range(2)
        if (h == 0 and c <= 9) or (h == 1 and c >= 6)]
BAND_IDX = {ch: j for j, ch in enumerate(BAND)}
NB = len(BAND)  # 20


def host_prep(inputs):
    """Returns (in_maps for 8 cores, postprocess-closure)."""
    x = np.asarray(inputs["x"], np.float32)
    Wq = np.asarray(inputs["Wq"], np.float32)
    Wk = np.asarray(inputs["Wk"], np.float32)
    Wv = np.asarray(inputs["Wv"], np.float32)
    Wo = np.asarray(inputs["Wo"], np.float32)
    bq = np.asarray(inputs["bq"], np.float32)
    bv = np.asarray(inputs["bv"], np.float32)
    bo = np.asarray(inputs["bo"], np.float32)
    et = float(np.asarray(inputs["et_gate"], np.float32).reshape(()))
    tb = np.asarray(inputs["time_bias"], np.float32).reshape(-1)
    assert tb.shape == (TB_LEN,)

    sig = np.float32(1.0 / (1.0 + np.exp(-et)))

    # banded E tiles: eb[p, j, u] = exp(sig*tb[clip(128c+p - q + 250)]),
    # q = 1024*half + u, for (c, half) = BAND[j]
    p = np.arange(128)[:, None]
    ebs = []
    for (c, h) in BAND:
        q = np.arange(h * 1024, h * 1024 + 1024)[None, :]
        idx = np.clip(c * 128 + p - q + TB_LEN // 2, 0, TB_LEN - 1)
        ebs.append(np.exp(sig * tb[idx]))
    eb = np.ascontiguousarray(np.stack(ebs, axis=1)).astype(ml_dtypes.bfloat16)

    # per-partition bias table for out-of-band exp folding
    btab = np.zeros((128, 2), np.float32)
    btab[:, 0] = sig * tb[0]           # half1, c<=5: idx clipped to 0
    btab[:, 1] = sig * tb[TB_LEN - 1]  # half0, c>=10: idx clipped to 499

    # wog[j, tf, do] = Wo.T reshaped (as v1)
    wg = np.ascontiguousarray(Wo.T.reshape(16, 64, D).transpose(1, 0, 2))
    wog = wg.astype(ml_dtypes.bfloat16)       # [64, 16, 1024]

    def chunk_w(Wl):  # Wl [256, 1024] -> [128, 8, 256]: [p, c, m] = Wl[m, c*128+p]
        return np.ascontiguousarray(
            Wl.T.reshape(NDC, 128, 256).transpose(1, 0, 2)
        ).astype(ml_dtypes.bfloat16)

    ones = np.ones((65, 512), np.float32)

    in_maps = []
    for core in range(8):
        b, hg = core // 4, core % 4
        sl = slice(hg * 256, (hg + 1) * 256)
        xc = np.ascontiguousarray(
            x[b].T.reshape(NDC, 128, T).transpose(1, 0, 2)
        ).astype(ml_dtypes.bfloat16)          # [128, 8, 2048]
        in_maps.append({
            "xc": xc,
            "wq": chunk_w(Wq[sl]),
            "wk": chunk_w(Wk[sl]),
            "wv": chunk_w(Wv[sl]),
            "wog": wog,
            "bqr": np.ascontiguousarray(bq[sl].reshape(1, 256)),
            "ones": ones,
            "eb": eb,
            "btab": btab,
        })

    corr = np.einsum("hj,jfd->hd", bv.reshape(16, HD), wg).astype(np.float32)

    def post(results):
        out = np.empty((2, T, D), np.float32)
        for core in range(8):
            b, hg = core // 4, core % 4
            yc = results[core]["y"]  # [512, 1024]
            for hl in range(4):
                h = hg * 4 + hl
                rows = (h % 8) * 256 + b * 128
                out[h // 8, rows:rows + 128, :] = (
                    yc[hl * 128:(hl + 1) * 128] + corr[h][None, :] + bo[None, :]
                )
        return out

    return in_maps, post


def expected_core(inputs, core):
    """Numpy model of one core's device output (for sim checks)."""
    m, _ = host_prep(inputs)
    im = m[core]
    xc = np.asarray(im["xc"], np.float32)      # [128, 8, 2048]
    xT = xc.transpose(1, 0, 2).reshape(D, T)   # [1024, 2048]
    eb = np.asarray(im["eb"], np.float32)
    btab = np.asarray(im["btab"], np.float32)
    # rebuild full E factor per (c, half)
    y = np.zeros((512, 1024), np.float32)
    wq = np.asarray(im["wq"], np.float32)
    wk = np.asarray(im["wk"], np.float32)
    wv = np.asarray(im["wv"], np.float32)
    Wq_l = np.concatenate([wq[:, c, :] for c in range(NDC)], axis=0)
    Wk_l = np.concatenate([wk[:, c, :] for c in range(NDC)], axis=0)
    Wv_l = np.concatenate([wv[:, c, :] for c in range(NDC)], axis=0)
    bqr = im["bqr"].reshape(256)
    xb = xT.astype(ml_dtypes.bfloat16).astype(np.float32)
    QT = Wq_l.T @ xb + bqr[:, None]
    KT = Wk_l.T @ xb
    QTb = QT.astype(ml_dtypes.bfloat16).astype(np.float32)
    KTb = KT.astype(ml_dtypes.bfloat16).astype(np.float32)
    V = (xb.T @ Wv_l).astype(ml_dtypes.bfloat16).astype(np.float32)
    wog = np.asarray(im["wog"], np.float32)
    for hl in range(4):
        qh = QTb[hl * 64:(hl + 1) * 64]
        kh = KTb[hl * 64:(hl + 1) * 64]
        P = np.zeros((T, T), np.float32)  # [k, q]
        sc = kh.T @ qh
        for c in range(NKC):
            for h2 in range(2):
                ksl = slice(c * 128, (c + 1) * 128)
                qsl = slice(h2 * 1024, (h2 + 1) * 1024)
                j = BAND_IDX.get((c, h2))
                if j is None:
                    bias = btab[0, 0] if h2 == 1 else btab[0, 1]
                    blk = np.exp(0.125 * sc[ksl, qsl] + bias)
                else:
                    blk = np.exp(0.125 * sc[ksl, qsl]) * eb[:, j, :]
                P[ksl, qsl] = blk.astype(ml_dtypes.bfloat16)
        c_ = (V[:, hl * 64:(hl + 1) * 64].T @ P) / P.sum(axis=0)[None, :]
        cn = c_.astype(ml_dtypes.bfloat16).astype(np.float32)
        g = cn.reshape(64, 128, 16)
        y[hl * 128:(hl + 1) * 128] = np.einsum("jcf,jfd->cd", g, wog)
    return y


def build_program(repeats=1):
    nc = bacc.Bacc("TRN2", target_bir_lowering=False, debug=False,
                   dynamic_dma_scratch_size=4096)
    xc_d = nc.dram_tensor("xc", [128, NDC, T], BF16, kind="ExternalInput").ap()
    wq_d = nc.dram_tensor("wq", [128, NDC, 256], BF16, kind="ExternalInput").ap()
    wk_d = nc.dram_tensor("wk", [128, NDC, 256], BF16, kind="ExternalInput").ap()
    wv_d = nc.dram_tensor("wv", [128, NDC, 256], BF16, kind="ExternalInput").ap()
    wog_d = nc.dram_tensor("wog", [64, 16, D], BF16, kind="ExternalInput").ap()
    bqr_d = nc.dram_tensor("bqr", [1, 256], F32R, kind="ExternalInput").ap()
    ones_d = nc.dram_tensor("ones", [65, 512], F32R, kind="ExternalInput").ap()
    eb_d = nc.dram_tensor("eb", [128, NB, 1024], BF16, kind="ExternalInput").ap()
    btab_d = nc.dram_tensor("btab", [128, 2], F32, kind="ExternalInput").ap()
    y_d = nc.dram_tensor("y", [512, D], F32, kind="ExternalOutput").ap()

    with tile.TileContext(nc) as tc:
        with (
            tc.tile_pool(name="const", bufs=1) as const,
            tc.tile_pool(name="persist", bufs=1) as persist,
            tc.tile_pool(name="pp", bufs=9) as pp,
            tc.tile_pool(name="ctxnp", bufs=2) as ctxnp,
            tc.tile_pool(name="rbp", bufs=2) as rbp,
            tc.tile_pool(name="csbp", bufs=2) as csbp,
            tc.tile_pool(name="bcp", bufs=2) as bcp,
            tc.tile_pool(name="yevac", bufs=2) as yevac,
            tc.tile_pool(name="scps", bufs=3, space="PSUM") as scps,
            tc.tile_pool(name="ctxps", bufs=2, space="PSUM") as ctxps,
        ):
            # ---- constants ----
            xsb = const.tile([128, NDC, T], BF16, tag="xsb")
            wq = const.tile([128, NDC, 256], BF16, tag="wq")
            wk = const.tile([128, NDC, 256], BF16, tag="wk")
            wv = const.tile([128, NDC, 256], BF16, tag="wv")
            wog = const.tile([64, 16, D], BF16, tag="wog")
            bqr = const.tile([1, 256], F32R, tag="bqr")
            ones_r = const.tile([65, 512], F32R, tag="ones_r")
            eb = const.tile([128, NB, 1024], BF16, tag="eb")
            btab = const.tile([128, 2], F32, tag="btab")

            nc.sync.dma_start(wq[:], wq_d[:])
            nc.sync.dma_start(wk[:], wk_d[:])
            nc.sync.dma_start(wv[:], wv_d[:])
            nc.sync.dma_start(bqr[:], bqr_d[:])
            nc.sync.dma_start(ones_r[:], ones_d[:])
            nc.sync.dma_start(btab[:], btab_d[:])
            for c in range(NDC):
                nc.sync.dma_start(xsb[:, c, :], xc_d[:, c, :])
            nc.sync.dma_start(wog[:], wog_d[:])
            for j in range(NB):
                nc.sync.dma_start(eb[:, j, :], eb_d[:, j, :])

            for _r in range(repeats):
                qT = [persist.tile([128, T], BF16, tag=f"qT{i}",
                                   name=f"qT{i}_{_r}") for i in range(2)]
                kT = [persist.tile([128, T], BF16, tag=f"kT{i}",
                                   name=f"kT{i}_{_r}") for i in range(2)]
                v_sb = persist.tile([128, NKC, 4, 65], BF16, tag="v_sb",
                                    name=f"v_sb_{_r}")
                nc.vector.memset(v_sb[:, :, :, 64:65], 1.0)
                # preload the exp table set off the critical path
                warm = rbp.tile([65, 512], F32, tag="rb", name=f"warm_{_r}")
                nc.scalar.activation(warm[0:1, 0:8], ones_r[0:1, 0:8].bitcast(F32), AF.Exp)
                # spin the PE while the x DMAs land so HAM un-throttles
                # (K=8/8) before the QKV projections begin
                warm_ps = scps.tile([128, 1024], F32, tag="sc",
                                    name=f"warmps_{_r}")
                for _w in range(44):
                    nc.tensor.matmul(warm_ps[0:64, 0:512],
                                     ones_r[0:1, 0:64], ones_r[0:1, :],
                                     start=True, stop=True)

                # ---- QKV projections ----
                for s in range(4):
                    q_ps = scps.tile([128, 2, 512], F32, tag="sc",
                                     name=f"qps_{_r}_{s}")
                    k_ps = scps.tile([128, 2, 512], F32, tag="sc",
                                     name=f"kps_{_r}_{s}")
                    for hp in range(2):
                        nc.tensor.matmul(
                            q_ps[:, hp, :], bqr[0:1, hp * 128:(hp + 1) * 128],
                            ones_r[0:1, :], start=True, stop=False)
                    for c in range(NDC):
                        xr = xsb[:, c, s * 512:(s + 1) * 512]
                        for hp in range(2):
                            nc.tensor.matmul(
                                q_ps[:, hp, :],
                                wq[:, c, hp * 128:(hp + 1) * 128],
                                xr, start=False, stop=(c == NDC - 1))
                            nc.tensor.matmul(
                                k_ps[:, hp, :],
                                wk[:, c, hp * 128:(hp + 1) * 128],
                                xr, start=(c == 0), stop=(c == NDC - 1))
                    for hp in range(2):
                        nc.scalar.activation(
                            qT[hp][:, s * 512:(s + 1) * 512], q_ps[:, hp, :],
                            AF.Copy)
                        nc.vector.tensor_copy(
                            kT[hp][:, s * 512:(s + 1) * 512], k_ps[:, hp, :])
                    for tp in range(2):
                        v_ps = scps.tile([128, 2, 512], F32, tag="sc",
                                         name=f"vps_{_r}_{s}_{tp}")
                        for c in range(NDC):
                            for g in range(2):
                                tb4 = tp * 2 + g
                                nc.tensor.matmul(
                                    v_ps[:, g, 0:256],
                                    xsb[:, c, s * 512 + tb4 * 128:
                                        s * 512 + (tb4 + 1) * 128],
                                    wv[:, c, :], start=(c == 0),
                                    stop=(c == NDC - 1))
                        for g in range(2):
                            kc = s * 4 + tp * 2 + g
                            nc.vector.tensor_copy(
                                v_sb[:, kc, :, 0:64],
                                v_ps[:, g, 0:256].rearrange(
                                    "p (h j) -> p h j", h=4))

                # ---- attention ----
                # Per head: two q-half passes (sp) of 1024 columns each.
                # AV emission lags scores by 4 chunks so the tensor FIFO
                # never blocks on the DVE mul queue; the previous pass's
                # normalize chain and the previous head's output projection
                # are spread across this pass's chunks as scheduled pieces
                # (recip@c1, broadcast@c2, nmul@c3, outproj@c9-c14), keeping
                # every engine FIFO free of long waits.
                pend = {}

                for hl in range(4):
                    hp, off = hl // 2, (hl % 2) * 64
                    ctxn = None
                    for sp in range(2):
                        pass_idx = hl * 2 + sp
                        base = pass_idx * 16
                        q0 = sp * 1024
                        cps = [None]

                        def emit_av(cc, pts, hl=hl, sp=sp, cps=cps, _r=_r):
                            if cps[0] is None:
                                cps[0] = [
                                    ctxps.tile([65, 512], F32, tag="ctx",
                                               name=f"cps{i}_{_r}_{hl}_{sp}")
                                    for i in range(2)]
                            pm = pts.pop(cc)
                            for si in range(2):
                                nc.tensor.matmul(
                                    cps[0][si][:],
                                    v_sb[:, cc, hl, :],
                                    pm[:, si * 512:(si + 1) * 512],
                                    start=(cc == 0), stop=(cc == NKC - 1))

                        pts = {}
                        for c in range(NKC):
                            for fn in pend.pop(base + c, []):
                                fn()
                            p_t = pp.tile([128, 1024], BF16, tag="p")
                            sc = scps.tile([128, 1024], F32, tag="sc")
                            for j in range(2):
                                nc.tensor.matmul(
                                    sc[:, j * 512:(j + 1) * 512],
                                    kT[hp][off:off + 64, c * 128:(c + 1) * 128],
                                    qT[hp][off:off + 64,
                                           q0 + j * 512:q0 + (j + 1) * 512],
                                    start=True, stop=True)
                            bi = BAND_IDX.get((c, sp))
                            if bi is None:
                                bias_ap = btab[:, 1:2] if sp == 0 else btab[:, 0:1]
                                nc.scalar.activation(p_t[:], sc[:], AF.Exp,
                                                     scale=0.125, bias=bias_ap)
                            else:
                                nc.scalar.activation(p_t[:], sc[:], AF.Exp,
                                                     scale=0.125)
                                nc.vector.tensor_mul(p_t[:], p_t[:],
                                                     eb[:, bi, :])
                            pts[c] = p_t
                            if c >= 4:
                                emit_av(c - 4, pts)

                        # ---- tail AVs + normalize: all deferred into the
                        # next pass's chunk slots so the PE pipeline never
                        # drains at a pass boundary ----
                        if ctxn is None:
                            ctxn = ctxnp.tile([64, T], BF16, tag="ctxn",
                                              name=f"ctxn_{_r}_{hl}")
                        srow = rbp.tile([65, 1024], F32, tag="srow",
                                        name=f"srow_{_r}_{hl}_{sp}")
                        ctx_sb = csbp.tile([64, 2, 512], F32, tag="csb",
                                           name=f"csb_{_r}_{hl}_{sp}")
                        packed = rbp.tile([64, 16], F32, tag="packed",
                                          name=f"packed_{_r}_{hl}_{sp}")

                        def tail_av(ccs, pts=pts, emit_av=emit_av):
                            for cc in ccs:
                                emit_av(cc, pts)

                        def norm_evac(cps=cps, srow=srow, ctx_sb=ctx_sb,
                                      packed=packed):
                            for si in range(2):
                                nc.vector.tensor_copy(
                                    srow[64:65, si * 512:(si + 1) * 512],
                                    cps[0][si][64:65, :])
                                nc.vector.tensor_copy(ctx_sb[:, si, :],
                                                      cps[0][si][0:64, :])
                            for si in range(2):
                                nc.sync.dma_start(
                                    packed[:, si * 8:(si + 1) * 8],
                                    srow[64:65, si * 512:(si + 1) * 512])

                        def norm_recip(hl=hl, sp=sp, _r=_r, packed=packed,
                                       store={}):
                            rp = rbp.tile([64, 16], F32R, tag="rp",
                                          name=f"rp_{_r}_{hl}_{sp}")
                            with nc.allow_low_precision(reason="f32r recip"):
                                nc.vector.reciprocal(rp[:], packed[:])
                            store['rp'] = rp
                            norm_recip.store = store

                        def norm_bcast(hl=hl, sp=sp, _r=_r, nr=norm_recip,
                                       store={}):
                            rp = nr.store['rp']
                            rb = rbp.tile([65, 1024], F32R, tag="rb",
                                          name=f"rb_{_r}_{hl}_{sp}")
                            bcs = []
                            for si in range(2):
                                nc.sync.dma_start(
                                    rb[64:65, si * 512:(si + 1) * 512],
                                    rp[:, si * 8:(si + 1) * 8])
                                bc_ps = scps.tile([64, 512], F32, tag="sc",
                                                  name=f"bcps_{_r}_{hl}_{sp}_{si}")
                                nc.tensor.matmul(
                                    bc_ps[:], ones_r[64:65, 0:64],
                                    rb[64:65, si * 512:(si + 1) * 512],
                                    start=True, stop=True)
                                bc_sb = bcp.tile([64, 512], F32, tag="bc",
                                                 name=f"bcsb_{_r}_{hl}_{sp}_{si}")
                                nc.vector.tensor_copy(bc_sb[:], bc_ps[:])
                                bcs.append(bc_sb)
                            norm_bcast.bcs = bcs

                        def norm_mul(hl=hl, sp=sp, q0=q0, ctx_sb=ctx_sb,
                                     ctxn=ctxn, nb=norm_bcast):
                            for si in range(2):
                                nc.vector.tensor_mul(
                                    ctxn[:, q0 + si * 512:q0 + (si + 1) * 512],
                                    ctx_sb[:, si, :], nb.bcs[si])

                        def boundary_fill(hp=hp):
                            for _f in range(8):
                                nc.tensor.ldweights(
                                    weights=kT[hp][0:64, 0:128])

                        nb = base + 16
                        pend.setdefault(nb + 0, []).append(boundary_fill)
                        pend.setdefault(nb + 0, []).append(
                            lambda t=tail_av: t((NKC - 4, NKC - 3)))
                        pend.setdefault(nb + 1, []).append(
                            lambda t=tail_av: t((NKC - 2,)))
                        pend.setdefault(nb + 2, []).append(
                            lambda t=tail_av: t((NKC - 1,)))
                        pend.setdefault(nb + 3, []).append(norm_evac)
                        pend.setdefault(nb + 5, []).append(norm_recip)
                        pend.setdefault(nb + 6, []).append(norm_bcast)
                        pend.setdefault(nb + 7, []).append(norm_mul)

                        if sp == 1:
                            def make_outproj(hl=hl, ctxn=ctxn, _r=_r):
                                ctxr = ctxn.rearrange("p (tc tf) -> p tf tc",
                                                      tf=16)
                                y_ps = [None]
                                fns = []
                                for ds in range(2):
                                    for half in range(2):
                                        def mm_fn(ds=ds, half=half):
                                            if half == 0:
                                                y_ps[0] = scps.tile(
                                                    [128, 512], F32, tag="sc",
                                                    name=f"yps{ds}_{_r}_{hl}")
                                            for tf in range(half * 8,
                                                            half * 8 + 8):
                                                nc.tensor.matmul(
                                                    y_ps[0][:], ctxr[:, tf, :],
                                                    wog[:, tf,
                                                        ds * 512:(ds + 1) * 512],
                                                    start=(tf == 0),
                                                    stop=(tf == 15))
                                        fns.append(mm_fn)

                                    def evac_fn(ds=ds):
                                        ysb = yevac.tile(
                                            [128, 512], F32, tag="y",
                                            name=f"ysb{ds}_{_r}_{hl}")
                                        nc.vector.tensor_copy(
                                            ysb[:], y_ps[0][:])
                                        nc.sync.dma_start(
                                            y_d[hl * 128:(hl + 1) * 128,
                                                ds * 512:(ds + 1) * 512],
                                            ysb[:])
                                    fns.append(evac_fn)
                                return fns

                            for ci, fn in zip(range(nb + 9, nb + 15),
                                              make_outproj()):
                                pend.setdefault(ci, []).append(fn)
                for c in sorted(pend):
                    for fn in pend[c]:
                        fn()
    nc.compile()
    return nc


_PROGRAM_CACHE = {}


def _get_program(repeats=1):
    if repeats not in _PROGRAM_CACHE:
        _PROGRAM_CACHE[repeats] = build_program(repeats=repeats)
    return _PROGRAM_CACHE[repeats]


def kernel(**inputs):
    from concourse.bass_utils import run_bass_kernel_spmd
    in_maps, post = host_prep(inputs)
    nc = _get_program(repeats=1)
    res = run_bass_kernel_spmd(nc, in_maps, list(range(8)))
    return post(res.results)


# revision 27
# speedup vs baseline: 1.0189x; 1.0189x over previous
"""BTSPAttention Trainium2 kernel for 8 NeuronCores (self-contained).

Usage: kernel(**inputs) -> np.ndarray  (full [2,2048,1024] float32 output)

Sharding: 8 cores = 2 batches x 4 head-groups (4 heads each).

v2 dataflow (vs v1 baseline at 546us; this version ~330-390us):
  - bf16 matmul operands everywhere (warm PE streams 2 bf16/cycle).
  - The clipped-Toeplitz time-bias multiply is skipped outside the +-250
    diagonal band: out-of-band [128k, 1024q] tiles fold the constant edge
    bias into the exp activation's bias operand (from a [128,2] SBUF
    table, so the program stays input-independent).
  - bk dropped entirely (contributes only q-constant scores terms, which
    softmax ignores); bq applied via a rank-1 start matmul.
  - Softmax reciprocals: sum rows are DMA-packed [1,1024]->[64,16] so one
    exact DVE reciprocal covers a whole pass at 16 elems/lane, scattered
    back (f32r) for the ones-stationary broadcast matmul.
  - Software pipelining tuned for the PE HAM clock gate (K=8/8 needs
    gap-free streams): a warm-up matmul spin covers the initial x DMA;
    per pass, AV emission lags scores by 4 chunks; each pass's tail AVs,
    normalize chain, and each head's output projection are deferred into
    the next pass's chunk slots (absolute-indexed pend schedule) so no
    engine FIFO ever blocks at a pass boundary.

Per head, per q-half pass (sp, 1024 cols), per k-chunk c (128 wide):
  scoresT[k,q] psum = kT_chunk.T @ qT      (2 N=512 bf16 matmuls)
  P = exp(0.125*scores + bias)             (ACT, [128,1024] per chunk)
  P *= eb[band]                            (DVE, banded chunks only)
  ctx[65, 2si, 512] psum += [V|1].T @ P    (row 64 = softmax sum)
Then: pack/recip/scatter + broadcast matmul + DVE normalize-mul -> ctxn
bf16; out rows are disjoint per head (faithful-torch 5-D transpose):
  y[hl*128 + tc, :] = sum_tf ctxn[:, 16tc+tf].T @ wog[:, tf, :]
Host folds: is_gate dropped (softmax shift-invariance); bv and bo applied
exactly on the host after the gather.
"""

import numpy as np
import ml_dtypes

import sys as _sys
if '/opt/trn_rl_repo' not in _sys.path:
    _sys.path.insert(0, '/opt/trn_rl_repo')


import concourse.bass as bass
import concourse.tile as tile
from concourse import bacc
from concourse import mybir

F32 = mybir.dt.float32
F32R = mybir.dt.float32r
BF16 = mybir.dt.bfloat16
AF = mybir.ActivationFunctionType

T = 2048
D = 1024
HD = 64
TB_LEN = 500
NKC = 16   # k chunks of 128
NDC = 8    # D chunks of 128

# (c, half) pairs whose [128k, 1024q] tile intersects the |k-q|<=250 band.
# half0 (q in [0,1024)): idx = 128c+p-q+250 varies iff c <= 9
# half1 (q in [1024,2048)): varies iff c >= 6
BAND = [(c, h) for c in range(NKC) for h in range(2)
        if (h == 0 and c <= 9) or (h == 1 and c >= 6)]
BAND_IDX = {ch: j for j, ch in enumerate(BAND)}
NB = len(BAND)  # 20


def host_prep(inputs):
    """Returns (in_maps for 8 cores, postprocess-closure)."""
    x = np.asarray(inputs["x"], np.float32)
    Wq = np.asarray(inputs["Wq"], np.float32)
    Wk = np.asarray(inputs["Wk"], np.float32)
    Wv = np.asarray(inputs["Wv"], np.float32)
    Wo = np.asarray(inputs["Wo"], np.float32)
    bq = np.asarray(inputs["bq"], np.float32)
    bv = np.asarray(inputs["bv"], np.float32)
    bo = np.asarray(inputs["bo"], np.float32)
    et = float(np.asarray(inputs["et_gate"], np.float32).reshape(()))
    tb = np.asarray(inputs["time_bias"], np.float32).reshape(-1)
    assert tb.shape == (TB_LEN,)

    sig = np.float32(1.0 / (1.0 + np.exp(-et)))

    # banded E tiles: eb[p, j, u] = exp(sig*tb[clip(128c+p - q + 250)]),
    # q = 1024*half + u, for (c, half) = BAND[j]
    p = np.arange(128)[:, None]
    ebs = []
    for (c, h) in BAND:
        q = np.arange(h * 1024, h * 1024 + 1024)[None, :]
        idx = np.clip(c * 128 + p - q + TB_LEN // 2, 0, TB_LEN - 1)
        ebs.append(np.exp(sig * tb[idx]))
    eb = np.ascontiguousarray(np.stack(ebs, axis=1)).astype(ml_dtypes.bfloat16)

    # per-partition bias table for out-of-band exp folding
    btab = np.zeros((128, 2), np.float32)
    btab[:, 0] = sig * tb[0]           # half1, c<=5: idx clipped to 0
    btab[:, 1] = sig * tb[TB_LEN - 1]  # half0, c>=10: idx clipped to 499

    # wog[j, tf, do] = Wo.T reshaped (as v1)
    wg = np.ascontiguousarray(Wo.T.reshape(16, 64, D).transpose(1, 0, 2))
    wog = wg.astype(ml_dtypes.bfloat16)       # [64, 16, 1024]

    def chunk_w(Wl):  # Wl [256, 1024] -> [128, 8, 256]: [p, c, m] = Wl[m, c*128+p]
        return np.ascontiguousarray(
            Wl.T.reshape(NDC, 128, 256).transpose(1, 0, 2)
        ).astype(ml_dtypes.bfloat16)

    ones = np.ones((65, 512), np.float32)

    in_maps = []
    for core in range(8):
        b, hg = core // 4, core % 4
        sl = slice(hg * 256, (hg + 1) * 256)
        xc = np.ascontiguousarray(
            x[b].T.reshape(NDC, 128, T).transpose(1, 0, 2)
        ).astype(ml_dtypes.bfloat16)          # [128, 8, 2048]
        in_maps.append({
            "xc": xc,
            "wq": chunk_w(Wq[sl]),
            "wk": chunk_w(Wk[sl]),
            "wv": chunk_w(Wv[sl]),
            "wog": wog,
            "bqr": np.ascontiguousarray(bq[sl].reshape(1, 256)),
            "ones": ones,
            "eb": eb,
            "btab": btab,
        })

    corr = np.einsum("hj,jfd->hd", bv.reshape(16, HD), wg).astype(np.float32)

    def post(results):
        out = np.empty((2, T, D), np.float32)
        for core in range(8):
            b, hg = core // 4, core % 4
            yc = results[core]["y"]  # [512, 1024]
            for hl in range(4):
                h = hg * 4 + hl
                rows = (h % 8) * 256 + b * 128
                out[h // 8, rows:rows + 128, :] = (
                    yc[hl * 128:(hl + 1) * 128] + corr[h][None, :] + bo[None, :]
                )
        return out

    return in_maps, post


def expected_core(inputs, core):
    """Numpy model of one core's device output (for sim checks)."""
    m, _ = host_prep(inputs)
    im = m[core]
    xc = np.asarray(im["xc"], np.float32)      # [128, 8, 2048]
    xT = xc.transpose(1, 0, 2).reshape(D, T)   # [1024, 2048]
    eb = np.asarray(im["eb"], np.float32)
    btab = np.asarray(im["btab"], np.float32)
    # rebuild full E factor per (c, half)
    y = np.zeros((512, 1024), np.float32)
    wq = np.asarray(im["wq"], np.float32)
    wk = np.asarray(im["wk"], np.float32)
    wv = np.asarray(im["wv"], np.float32)
    Wq_l = np.concatenate([wq[:, c, :] for c in range(NDC)], axis=0)
    Wk_l = np.concatenate([wk[:, c, :] for c in range(NDC)], axis=0)
    Wv_l = np.concatenate([wv[:, c, :] for c in range(NDC)], axis=0)
    bqr = im["bqr"].reshape(256)
    xb = xT.astype(ml_dtypes.bfloat16).astype(np.float32)
    QT = Wq_l.T @ xb + bqr[:, None]
    KT = Wk_l.T @ xb
    QTb = QT.astype(ml_dtypes.bfloat16).astype(np.float32)
    KTb = KT.astype(ml_dtypes.bfloat16).astype(np.float32)
    V = (xb.T @ Wv_l).astype(ml_dtypes.bfloat16).astype(np.float32)
    wog = np.asarray(im["wog"], np.float32)
    for hl in range(4):
        qh = QTb[hl * 64:(hl + 1) * 64]
        kh = KTb[hl * 64:(hl + 1) * 64]
        P = np.zeros((T, T), np.float32)  # [k, q]
        sc = kh.T @ qh
        for c in range(NKC):
            for h2 in range(2):
                ksl = slice(c * 128, (c + 1) * 128)
                qsl = slice(h2 * 1024, (h2 + 1) * 1024)
                j = BAND_IDX.get((c, h2))
                if j is None:
                    bias = btab[0, 0] if h2 == 1 else btab[0, 1]
                    blk = np.exp(0.125 * sc[ksl, qsl] + bias)
                else:
                    blk = np.exp(0.125 * sc[ksl, qsl]) * eb[:, j, :]
                P[ksl, qsl] = blk.astype(ml_dtypes.bfloat16)
        c_ = (V[:, hl * 64:(hl + 1) * 64].T @ P) / P.sum(axis=0)[None, :]
        cn = c_.astype(ml_dtypes.bfloat16).astype(np.float32)
        g = cn.reshape(64, 128, 16)
        y[hl * 128:(hl + 1) * 128] = np.einsum("jcf,jfd->cd", g, wog)
    return y


def build_program(repeats=1):
    nc = bacc.Bacc("TRN2", target_bir_lowering=False, debug=False,
                   dynamic_dma_scratch_size=4096)
    xc_d = nc.dram_tensor("xc", [128, NDC, T], BF16, kind="ExternalInput").ap()
    wq_d = nc.dram_tensor("wq", [128, NDC, 256], BF16, kind="ExternalInput").ap()
    wk_d = nc.dram_tensor("wk", [128, NDC, 256], BF16, kind="ExternalInput").ap()
    wv_d = nc.dram_tensor("wv", [128, NDC, 256], BF16, kind="ExternalInput").ap()
    wog_d = nc.dram_tensor("wog", [64, 16, D], BF16, kind="ExternalInput").ap()
    bqr_d = nc.dram_tensor("bqr", [1, 256], F32R, kind="ExternalInput").ap()
    ones_d = nc.dram_tensor("ones", [65, 512], F32R, kind="ExternalInput").ap()
    eb_d = nc.dram_tensor("eb", [128, NB, 1024], BF16, kind="ExternalInput").ap()
    btab_d = nc.dram_tensor("btab", [128, 2], F32, kind="ExternalInput").ap()
    y_d = nc.dram_tensor("y", [512, D], F32, kind="ExternalOutput").ap()

    with tile.TileContext(nc) as tc:
        with (
            tc.tile_pool(name="const", bufs=1) as const,
            tc.tile_pool(name="persist", bufs=1) as persist,
            tc.tile_pool(name="pp", bufs=9) as pp,
            tc.tile_pool(name="ctxnp", bufs=2) as ctxnp,
            tc.tile_pool(name="rbp", bufs=2) as rbp,
            tc.tile_pool(name="csbp", bufs=2) as csbp,
            tc.tile_pool(name="bcp", bufs=2) as bcp,
            tc.tile_pool(name="yevac", bufs=2) as yevac,
            tc.tile_pool(name="scps", bufs=3, space="PSUM") as scps,
            tc.tile_pool(name="ctxps", bufs=2, space="PSUM") as ctxps,
        ):
            # ---- constants ----
            xsb = const.tile([128, NDC, T], BF16, tag="xsb")
            wq = const.tile([128, NDC, 256], BF16, tag="wq")
            wk = const.tile([128, NDC, 256], BF16, tag="wk")
            wv = const.tile([128, NDC, 256], BF16, tag="wv")
            wog = const.tile([64, 16, D], BF16, tag="wog")
            bqr = const.tile([1, 256], F32R, tag="bqr")
            ones_r = const.tile([65, 512], F32R, tag="ones_r")
            eb = const.tile([128, NB, 1024], BF16, tag="eb")
            btab = const.tile([128, 2], F32, tag="btab")

            nc.sync.dma_start(wq[:], wq_d[:])
            nc.sync.dma_start(wk[:], wk_d[:])
            nc.sync.dma_start(wv[:], wv_d[:])
            nc.sync.dma_start(bqr[:], bqr_d[:])
            nc.sync.dma_start(ones_r[:], ones_d[:])
            nc.sync.dma_start(btab[:], btab_d[:])
            for c in range(NDC):
                nc.sync.dma_start(xsb[:, c, :], xc_d[:, c, :])
            nc.sync.dma_start(wog[:], wog_d[:])
            for j in range(NB):
                nc.sync.dma_start(eb[:, j, :], eb_d[:, j, :])

            for _r in range(repeats):
                qT = [persist.tile([128, T], BF16, tag=f"qT{i}",
                                   name=f"qT{i}_{_r}") for i in range(2)]
                kT = [persist.tile([128, T], BF16, tag=f"kT{i}",
                                   name=f"kT{i}_{_r}") for i in range(2)]
                v_sb = persist.tile([128, NKC, 4, 65], BF16, tag="v_sb",
                                    name=f"v_sb_{_r}")
                nc.vector.memset(v_sb[:, :, :, 64:65], 1.0)
                # preload the exp table set off the critical path
                warm = rbp.tile([65, 512], F32, tag="rb", name=f"warm_{_r}")
                nc.scalar.activation(warm[0:1, 0:8], ones_r[0:1, 0:8].bitcast(F32), AF.Exp)
                # spin the PE while the x DMAs land so HAM un-throttles
                # (K=8/8) before the QKV projections begin
                warm_ps = scps.tile([128, 1024], F32, tag="sc",
                                    name=f"warmps_{_r}")
                for _w in range(24):
                    nc.tensor.matmul(warm_ps[0:64, 0:512],
                                     ones_r[0:1, 0:64], ones_r[0:1, :],
                                     start=True, stop=True)

                # ---- QKV projections ----
                for s in range(4):
                    q_ps = scps.tile([128, 2, 512], F32, tag="sc",
                                     name=f"qps_{_r}_{s}")
                    k_ps = scps.tile([128, 2, 512], F32, tag="sc",
                                     name=f"kps_{_r}_{s}")
                    for hp in range(2):
                        nc.tensor.matmul(
                            q_ps[:, hp, :], bqr[0:1, hp * 128:(hp + 1) * 128],
                            ones_r[0:1, :], start=True, stop=False)
                    for c in range(NDC):
                        xr = xsb[:, c, s * 512:(s + 1) * 512]
                        for hp in range(2):
                            nc.tensor.matmul(
                                q_ps[:, hp, :],
                                wq[:, c, hp * 128:(hp + 1) * 128],
                                xr, start=False, stop=(c == NDC - 1))
                            nc.tensor.matmul(
                                k_ps[:, hp, :],
                                wk[:, c, hp * 128:(hp + 1) * 128],
                                xr, start=(c == 0), stop=(c == NDC - 1))
                    for hp in range(2):
                        nc.scalar.activation(
                            qT[hp][:, s * 512:(s + 1) * 512], q_ps[:, hp, :],
                            AF.Copy)
                        nc.vector.tensor_copy(
                            kT[hp][:, s * 512:(s + 1) * 512], k_ps[:, hp, :])
                    for tp in range(2):
                        v_ps = scps.tile([128, 2, 512], F32, tag="sc",
                                         name=f"vps_{_r}_{s}_{tp}")
                        for c in range(NDC):
                            for g in range(2):
                                tb4 = tp * 2 + g
                                nc.tensor.matmul(
                                    v_ps[:, g, 0:256],
                                    xsb[:, c, s * 512 + tb4 * 128:
                                        s * 512 + (tb4 + 1) * 128],
                                    wv[:, c, :], start=(c == 0),
                                    stop=(c == NDC - 1))
                        for g in range(2):
                            kc = s * 4 + tp * 2 + g
                            nc.vector.tensor_copy(
                                v_sb[:, kc, :, 0:64],
                                v_ps[:, g, 0:256].rearrange(
                                    "p (h j) -> p h j", h=4))

                # ---- attention ----
                # Per head: two q-half passes (sp) of 1024 columns each.
                # AV emission lags scores by 4 chunks so the tensor FIFO
                # never blocks on the DVE mul queue; the previous pass's
                # normalize chain and the previous head's output projection
                # are spread across this pass's chunks as scheduled pieces
                # (recip@c1, broadcast@c2, nmul@c3, outproj@c9-c14), keeping
                # every engine FIFO free of long waits.
                pend = {}

                for hl in range(4):
                    hp, off = hl // 2, (hl % 2) * 64
                    ctxn = None
                    for sp in range(2):
                        pass_idx = hl * 2 + sp
                        base = pass_idx * 16
                        q0 = sp * 1024
                        cps = [None]

                        def emit_av(cc, pts, hl=hl, sp=sp, cps=cps, _r=_r):
                            if cps[0] is None:
                                cps[0] = [
                                    ctxps.tile([65, 512], F32, tag="ctx",
                                               name=f"cps{i}_{_r}_{hl}_{sp}")
                                    for i in range(2)]
                            pm = pts.pop(cc)
                            for si in range(2):
                                nc.tensor.matmul(
                                    cps[0][si][:],
                                    v_sb[:, cc, hl, :],
                                    pm[:, si * 512:(si + 1) * 512],
                                    start=(cc == 0), stop=(cc == NKC - 1))

                        pts = {}
                        for c in range(NKC):
                            for fn in pend.pop(base + c, []):
                                fn()
                            p_t = pp.tile([128, 1024], BF16, tag="p")
                            sc = scps.tile([128, 1024], F32, tag="sc")
                            for j in range(2):
                                nc.tensor.matmul(
                                    sc[:, j * 512:(j + 1) * 512],
                                    kT[hp][off:off + 64, c * 128:(c + 1) * 128],
                                    qT[hp][off:off + 64,
                                           q0 + j * 512:q0 + (j + 1) * 512],
                                    start=True, stop=True)
                            bi = BAND_IDX.get((c, sp))
                            if bi is None:
                                bias_ap = btab[:, 1:2] if sp == 0 else btab[:, 0:1]
                                nc.scalar.activation(p_t[:], sc[:], AF.Exp,
                                                     scale=0.125, bias=bias_ap)
                            else:
                                nc.scalar.activation(p_t[:], sc[:], AF.Exp,
                                                     scale=0.125)
                                nc.vector.tensor_mul(p_t[:], p_t[:],
                                                     eb[:, bi, :])
                            pts[c] = p_t
                            if c >= 4:
                                emit_av(c - 4, pts)

                        # ---- tail AVs + normalize: all deferred into the
                        # next pass's chunk slots so the PE pipeline never
                        # drains at a pass boundary ----
                        if ctxn is None:
                            ctxn = ctxnp.tile([64, T], BF16, tag="ctxn",
                                              name=f"ctxn_{_r}_{hl}")
                        srow = rbp.tile([65, 1024], F32, tag="srow",
                                        name=f"srow_{_r}_{hl}_{sp}")
                        ctx_sb = csbp.tile([64, 2, 512], F32, tag="csb",
                                           name=f"csb_{_r}_{hl}_{sp}")
                        packed = rbp.tile([64, 16], F32, tag="packed",
                                          name=f"packed_{_r}_{hl}_{sp}")

                        def tail_av(ccs, pts=pts, emit_av=emit_av):
                            for cc in ccs:
                                emit_av(cc, pts)

                        def norm_evac(cps=cps, srow=srow, ctx_sb=ctx_sb,
                                      packed=packed):
                            for si in range(2):
                                nc.vector.tensor_copy(
                                    srow[64:65, si * 512:(si + 1) * 512],
                                    cps[0][si][64:65, :])
                                nc.vector.tensor_copy(ctx_sb[:, si, :],
                                                      cps[0][si][0:64, :])
                            for si in range(2):
                                nc.sync.dma_start(
                                    packed[:, si * 8:(si + 1) * 8],
                                    srow[64:65, si * 512:(si + 1) * 512])

                        def norm_recip(hl=hl, sp=sp, _r=_r, packed=packed,
                                       store={}):
                            rp = rbp.tile([64, 16], F32R, tag="rp",
                                          name=f"rp_{_r}_{hl}_{sp}")
                            with nc.allow_low_precision(reason="f32r recip"):
                                nc.vector.reciprocal(rp[:], packed[:])
                            store['rp'] = rp
                            norm_recip.store = store

                        def norm_bcast(hl=hl, sp=sp, _r=_r, nr=norm_recip,
                                       store={}):
                            rp = nr.store['rp']
                            rb = rbp.tile([65, 1024], F32R, tag="rb",
                                          name=f"rb_{_r}_{hl}_{sp}")
                            bcs = []
                            for si in range(2):
                                nc.sync.dma_start(
                                    rb[64:65, si * 512:(si + 1) * 512],
                                    rp[:, si * 8:(si + 1) * 8])
                                bc_ps = scps.tile([64, 512], F32, tag="sc",
                                                  name=f"bcps_{_r}_{hl}_{sp}_{si}")
                                nc.tensor.matmul(
                                    bc_ps[:], ones_r[64:65, 0:64],
                                    rb[64:65, si * 512:(si + 1) * 512],
                                    start=True, stop=True)
                                bc_sb = bcp.tile([64, 512], F32, tag="bc",
                                                 name=f"bcsb_{_r}_{hl}_{sp}_{si}")
                                nc.vector.tensor_copy(bc_sb[:], bc_ps[:])
                                bcs.append(bc_sb)
                            norm_bcast.bcs = bcs

                        def norm_mul(hl=hl, sp=sp, q0=q0, ctx_sb=ctx_sb,
                                     ctxn=ctxn, nb=norm_bcast):
                            for si in range(2):
                                nc.vector.tensor_mul(
                                    ctxn[:, q0 + si * 512:q0 + (si + 1) * 512],
                                    ctx_sb[:, si, :], nb.bcs[si])

                        def boundary_fill(hp=hp):
                            for _f in range(8):
                                nc.tensor.ldweights(
                                    weights=kT[hp][0:64, 0:128])

                        nb = base + 16
                        pend.setdefault(nb + 0, []).append(boundary_fill)
                        pend.setdefault(nb + 0, []).append(
                            lambda t=tail_av: t((NKC - 4, NKC - 3)))
                        pend.setdefault(nb + 1, []).append(
                            lambda t=tail_av: t((NKC - 2,)))
                        pend.setdefault(nb + 2, []).append(
                            lambda t=tail_av: t((NKC - 1,)))
                        pend.setdefault(nb + 3, []).append(norm_evac)
                        pend.setdefault(nb + 5, []).append(norm_recip)
                        pend.setdefault(nb + 6, []).append(norm_bcast)
                        pend.setdefault(nb + 7, []).append(norm_mul)

                        if sp == 1:
                            def make_outproj(hl=hl, ctxn=ctxn, _r=_r):
                                ctxr = ctxn.rearrange("p (tc tf) -> p tf tc",
                                                      tf=16)
                                y_ps = [None]
                                fns = []
                                for ds in range(2):
                                    for half in range(2):
                                        def mm_fn(ds=ds, half=half):
                                            if half == 0:
                                                y_ps[0] = scps.tile(
                                                    [128, 512], F32, tag="sc",
                                                    name=f"yps{ds}_{_r}_{hl}")
                                            for tf in range(half * 8,
                                                            half * 8 + 8):
                                                nc.tensor.matmul(
                                                    y_ps[0][:], ctxr[:, tf, :],
                                                    wog[:, tf,
                                                        ds * 512:(ds + 1) * 512],
                                                    start=(tf == 0),
                                                    stop=(tf == 15))
                                        fns.append(mm_fn)

                                    def evac_fn(ds=ds):
                                        ysb = yevac.tile(
                                            [128, 512], F32, tag="y",
                                            name=f"ysb{ds}_{_r}_{hl}")
                                        nc.vector.tensor_copy(
                                            ysb[:], y_ps[0][:])
                                        nc.sync.dma_start(
                                            y_d[hl * 128:(hl + 1) * 128,
                                                ds * 512:(ds + 1) * 512],
                                            ysb[:])
                                    fns.append(evac_fn)
                                return fns

                            for ci, fn in zip(range(nb + 9, nb + 15),
                                              make_outproj()):
                                pend.setdefault(ci, []).append(fn)
                for c in sorted(pend):
                    for fn in pend[c]:
                        fn()
    nc.compile()
    return nc


_PROGRAM_CACHE = {}


def _get_program(repeats=1):
    if repeats not in _PROGRAM_CACHE:
        _PROGRAM_CACHE[repeats] = build_program(repeats=repeats)
    return _PROGRAM_CACHE[repeats]


def kernel(**inputs):
    from concourse.bass_utils import run_bass_kernel_spmd
    in_maps, post = host_prep(inputs)
    nc = _get_program(repeats=1)
    res = run_bass_kernel_spmd(nc, in_maps, list(range(8)))
    return post(res.results)
